# revision 1
# baseline (speedup 1.0000x reference)
"""Adaptive-softmax NLL on 8 TRN2 NeuronCores (Bass/Tile, SPMD data-parallel).

Strategy: shard the 4096 tokens across 8 cores (512 each). Each core computes
its tokens' full NLL (head + both tails) in bf16 on TensorE, with vocab on the
free dim and tokens on PSUM partitions; ScalarE does exp with fused free-dim
accumulation (accum_out) so the softmax denominators come out of the same pass.
Target logits are computed from host-gathered weight columns (MoE-style
dispatch done at input-prep time) as elementwise-mul + ones-matvec partition
reductions. Each core emits one partial-loss scalar; the host sums 8 scalars
and divides by N.
"""

import os
import sys
import types

import numpy as np
import ml_dtypes

BF16 = ml_dtypes.bfloat16

# ---- problem constants (hardcoded; kernel.py must be self-contained) ----
CUTOFF = [4000, 20000, 50000]
D = 1024
N = 4096
NCORES = 8
TOK = N // NCORES          # 512 tokens per core
NT = TOK // 128            # 4 token tiles of 128
HEAD_V = CUTOFF[0] + 2     # 4002
T0_V = CUTOFF[1] - CUTOFF[0]   # 16000
T1_V = CUTOFF[2] - CUTOFF[1]   # 30000
D1 = D // 4                # 256 tail1 bottleneck


def _chunks(v):
    out = []
    while v > 0:
        out.append(min(512, v))
        v -= out[-1]
    return out


H_CH = _chunks(HEAD_V)     # [512]*7 + [418]
T0_CH = _chunks(T0_V)      # [512]*31 + [128]
T1_CH = _chunks(T1_V)      # [512]*58 + [296]

LAST_EXEC_NS = None
_CACHE = {}


def _install_axon_profile_shim():
    """The image's antenv lacks axon_hooks; register the NTFF hook + disable
    the FishPath artifact upload so BASS_TRACE=1 profiling works locally."""
    if "antenv.axon_hooks" not in sys.modules:
        try:
            import antenv  # noqa
            mod = types.ModuleType("antenv.axon_hooks")
            _hook = [None]
            mod.set_axon_ntff_profile_hook = lambda h: _hook.__setitem__(0, h)
            mod.get_axon_ntff_profile_hook = lambda: _hook[0]
            sys.modules["antenv.axon_hooks"] = mod
            antenv.axon_hooks = mod
            from trn_agent_boot.trn_boot import _ntff_profile_via_ctypes
            mod.set_axon_ntff_profile_hook(
                _ntff_profile_via_ctypes("/opt/axon/libaxon_pjrt.so")
            )
        except Exception:
            pass
    try:
        from concourse import bass_utils
        bass_utils.upload_artifacts = lambda tmpdir: f"local:{tmpdir}"
    except Exception:
        pass


# ---------------- host-side layout helpers ----------------

def _tile_k(w):
    """[K, M] f32 -> [128, K//128, M] bf16 (partition, k-tile, free)."""
    K, M = w.shape
    kd = K // 128
    return np.ascontiguousarray(
        w.reshape(kd, 128, M).transpose(1, 0, 2)
    ).astype(BF16)


def _chunk_weights(w, chunk_sizes):
    """[K, V] f32 -> [nchunk, 128, K//128, 512] bf16, zero-padded ragged."""
    K, V = w.shape
    kd = K // 128
    out = np.zeros((len(chunk_sizes), 128, kd, 512), dtype=BF16)
    c0 = 0
    for i, ncs in enumerate(chunk_sizes):
        blk = w[:, c0:c0 + ncs].reshape(kd, 128, ncs).transpose(1, 0, 2)
        out[i, :, :, :ncs] = blk.astype(BF16)
        c0 += ncs
    return out


# ---------------- device kernel builder ----------------

def _build():
    from concourse import bass, bacc, tile

    mybir = bass.mybir
    dt = mybir.dt
    bf = dt.bfloat16
    f32 = dt.float32
    AF = mybir.ActivationFunctionType
    ALU = mybir.AluOpType
    AX = mybir.AxisListType

    nc = bacc.Bacc(
        "TRN2",
        target_bir_lowering=False,
        debug=False,
        enable_asserts=False,
        num_devices=NCORES,
    )

    def din(name, shape, dtype=bf):
        return nc.dram_tensor(name, list(shape), dtype, kind="ExternalInput")

    wiT_h = din("wiT", (128, 8, TOK))
    selH_h = din("selH", (128, 8, TOK))
    sel0_h = din("sel0", (128, 8, TOK))
    sel1_h = din("sel1", (128, 2, TOK))
    bsel_h = din("bsel", (1, TOK))
    m0_h = din("m0", (128, NT), f32)
    m1_h = din("m1", (128, NT), f32)
    bext_h = din("bext", (1, HEAD_V))
    hw_h = din("hw", (len(H_CH), 128, 8, 512))
    w20_h = din("w20", (len(T0_CH), 128, 8, 512))
    w21_h = din("w21", (len(T1_CH), 128, 2, 512))
    w10_h = din("w10", (128, 8, D))
    w11_h = din("w11", (128, 8, D1))
    out_h = nc.dram_tensor("out", [1, 1], f32, kind="ExternalOutput")

    with tile.TileContext(nc) as tc:
        with (
            tc.tile_pool(name="const", bufs=1) as cpool,
            tc.tile_pool(name="wstream", bufs=4) as wpool,
            tc.tile_pool(name="scratch", bufs=3) as spool,
            tc.tile_pool(name="pmm", bufs=4, space=bass.MemorySpace.PSUM) as pmm,
            tc.tile_pool(name="prow", bufs=1, space=bass.MemorySpace.PSUM) as prow,
        ):
            wiT = cpool.tile([128, 8, TOK], bf)
            w10 = cpool.tile([128, 8, D], bf)
            w11 = cpool.tile([128, 8, D1], bf)
            selH = cpool.tile([128, 8, TOK], bf)
            sel0 = cpool.tile([128, 8, TOK], bf)
            sel1 = cpool.tile([128, 2, TOK], bf)
            bsel = cpool.tile([1, TOK], bf)
            m0sb = cpool.tile([128, NT], f32)
            m1sb = cpool.tile([128, NT], f32)
            bext = cpool.tile([1, HEAD_V], bf)
            h0T = cpool.tile([128, 8, TOK], bf)
            h1T = cpool.tile([128, 2, TOK], bf)
            seH = cpool.tile([128, NT, len(H_CH)], f32)
            se0 = cpool.tile([128, NT, len(T0_CH)], f32)
            se1 = cpool.tile([128, NT, len(T1_CH)], f32)
            ones_col = cpool.tile([128, 1], bf)
            ones_row = cpool.tile([1, 128], bf)
            onesf_col = cpool.tile([128, 1], f32)

            # resident loads (weights first; sel* only needed at the end)
            nc.sync.dma_start(out=wiT[:], in_=wiT_h[:])
            nc.sync.dma_start(out=w10[:], in_=w10_h[:])
            nc.sync.dma_start(out=w11[:], in_=w11_h[:])
            nc.sync.dma_start(out=bext[:], in_=bext_h[:])
            nc.sync.dma_start(out=m0sb[:], in_=m0_h[:])
            nc.sync.dma_start(out=m1sb[:], in_=m1_h[:])
            nc.sync.dma_start(out=bsel[:], in_=bsel_h[:])
            nc.vector.memset(ones_col[:], 1.0)
            nc.vector.memset(ones_row[:], 1.0)
            nc.vector.memset(onesf_col[:], 1.0)

            def softmax_block(wh, nk, lhsT, chunk_sizes, se, bias=None, tag="w8"):
                c0 = 0
                for c, ncs in enumerate(chunk_sizes):
                    wt = wpool.tile([128, nk, 512], bf, tag=tag)
                    nc.sync.dma_start(out=wt[:], in_=wh.ap()[c])
                    for jt in range(NT):
                        ps = pmm.tile([128, 512], f32)
                        for k in range(nk):
                            nc.tensor.matmul(
                                ps[:, :ncs],
                                lhsT[:, k, jt * 128:(jt + 1) * 128],
                                wt[:, k, :ncs],
                                start=(k == 0),
                                stop=(k == nk - 1 and bias is None),
                            )
                        if bias is not None:
                            nc.tensor.matmul(
                                ps[:, :ncs],
                                ones_row[:, :],
                                bias[:, c0:c0 + ncs],
                                start=False,
                                stop=True,
                            )
                        ex = spool.tile([128, 512], bf, tag="expout")
                        nc.scalar.activation(
                            ex[:, :ncs],
                            ps[:, :ncs],
                            AF.Exp,
                            accum_out=se[:, jt, c:c + 1],
                        )
                    c0 += ncs

            # head CE over remapped targets (uses w_in directly)
            softmax_block(hw_h, 8, wiT, H_CH, seH, bias=bext)

            # h0T = (w_in @ tail0_w1)^T  [1024, 512] bf16
            for m in range(8):
                ps = pmm.tile([128, 512], f32)
                for k in range(8):
                    nc.tensor.matmul(
                        ps[:],
                        w10[:, k, m * 128:(m + 1) * 128],
                        wiT[:, k, :],
                        start=(k == 0),
                        stop=(k == 7),
                    )
                nc.vector.tensor_copy(h0T[:, m, :], ps[:])

            softmax_block(w20_h, 8, h0T, T0_CH, se0)

            # h1T = (w_in @ tail1_w1)^T  [256, 512] bf16
            for m in range(2):
                ps = pmm.tile([128, 512], f32)
                for k in range(8):
                    nc.tensor.matmul(
                        ps[:],
                        w11[:, k, m * 128:(m + 1) * 128],
                        wiT[:, k, :],
                        start=(k == 0),
                        stop=(k == 7),
                    )
                nc.vector.tensor_copy(h1T[:, m, :], ps[:])

            softmax_block(w21_h, 2, h1T, T1_CH, se1, tag="w2")

            # late resident loads for the target-logit dot products
            nc.sync.dma_start(out=selH[:], in_=selH_h[:])
            nc.sync.dma_start(out=sel0[:], in_=sel0_h[:])
            nc.sync.dma_start(out=sel1[:], in_=sel1_h[:])

            # target logits: sum_d a[d,t]*sel[d,t] via DVE mul + ones matvec,
            # all accumulated into one PSUM row [1, TOK]
            rowp = prow.tile([1, TOK], f32)
            pieces = [(wiT, selH, 8), (h0T, sel0, 8), (h1T, sel1, 2)]
            total = sum(nk for _, _, nk in pieces)
            i = 0
            for a, b, nk in pieces:
                for k in range(nk):
                    mt = spool.tile([128, TOK], bf, tag="mul")
                    nc.vector.tensor_mul(mt[:], a[:, k, :], b[:, k, :])
                    nc.tensor.matmul(
                        rowp[:], ones_col[:], mt[:], start=(i == 0), stop=False
                    )
                    i += 1
            nc.tensor.matmul(
                rowp[:], ones_row[:, 0:1], bsel[:], start=False, stop=True
            )

            # per-token log-sum-exp -> masked sum
            seH_r = cpool.tile([128, NT], f32)
            se0_r = cpool.tile([128, NT], f32)
            se1_r = cpool.tile([128, NT], f32)
            nc.vector.tensor_reduce(seH_r[:], seH[:], AX.X, ALU.add)
            nc.vector.tensor_reduce(se0_r[:], se0[:], AX.X, ALU.add)
            nc.vector.tensor_reduce(se1_r[:], se1[:], AX.X, ALU.add)
            logH = cpool.tile([128, NT], f32)
            log0 = cpool.tile([128, NT], f32)
            log1 = cpool.tile([128, NT], f32)
            nc.scalar.activation(logH[:], seH_r[:], AF.Ln)
            nc.scalar.activation(log0[:], se0_r[:], AF.Ln)
            nc.scalar.activation(log1[:], se1_r[:], AF.Ln)
            log0m = cpool.tile([128, NT], f32)
            log1m = cpool.tile([128, NT], f32)
            nc.vector.tensor_mul(log0m[:], log0[:], m0sb[:])
            nc.vector.tensor_mul(log1m[:], log1[:], m1sb[:])
            acc = cpool.tile([128, NT], f32)
            nc.vector.tensor_add(acc[:], logH[:], log0m[:])
            nc.vector.tensor_add(acc[:], acc[:], log1m[:])

            pf = prow.tile([1, NT], f32, tag="pf")
            nc.tensor.matmul(pf[:], onesf_col[:], acc[:], start=True, stop=True)

            logsum = cpool.tile([1, 1], f32)
            tgts = cpool.tile([1, 1], f32)
            res = cpool.tile([1, 1], f32)
            nc.vector.tensor_reduce(logsum[:], pf[:], AX.X, ALU.add)
            nc.vector.tensor_reduce(tgts[:], rowp[:], AX.X, ALU.add)
            nc.vector.tensor_sub(res[:], logsum[:], tgts[:])
            nc.sync.dma_start(out=out_h[:], in_=res[:])

    nc.compile()
    return nc


# ---------------- entry point ----------------

def kernel(**inputs):
    global LAST_EXEC_NS
    _install_axon_profile_shim()
    from concourse import bass_utils

    w_in = np.asarray(inputs["w_in"], dtype=np.float32)
    target = np.asarray(inputs["target"], dtype=np.int64)
    head_w = np.asarray(inputs["head_w"], dtype=np.float32)
    head_b = np.asarray(inputs["head_b"], dtype=np.float32)
    t0w1 = np.asarray(inputs["tail0_w1"], dtype=np.float32)
    t0w2 = np.asarray(inputs["tail0_w2"], dtype=np.float32)
    t1w1 = np.asarray(inputs["tail1_w1"], dtype=np.float32)
    t1w2 = np.asarray(inputs["tail1_w2"], dtype=np.float32)

    # target-derived bookkeeping (pure indexing, part of input sharding)
    m0 = (target >= CUTOFF[0]) & (target < CUTOFF[1])
    m1 = (target >= CUTOFF[1]) & (target < CUTOFF[2])
    first_target = np.where(m0, CUTOFF[0], np.where(m1, CUTOFF[0] + 1, target))
    idx0 = np.clip(target - CUTOFF[0], 0, T0_V - 1)
    idx1 = np.clip(target - CUTOFF[1], 0, T1_V - 1)

    # shared (replicated) weight payloads, laid out as their SBUF images
    shared = {
        "bext": head_b[None, :].astype(BF16),
        "hw": _chunk_weights(head_w, H_CH),
        "w20": _chunk_weights(t0w2, T0_CH),
        "w21": _chunk_weights(t1w2, T1_CH),
        "w10": _tile_k(t0w1),
        "w11": _tile_k(t1w1),
    }

    wiT = w_in.T  # [D, N]
    selH_all = head_w[:, first_target]            # [D, N]
    sel0_all = t0w2[:, idx0] * m0[None, :]        # [D, N] masked
    sel1_all = t1w2[:, idx1] * m1[None, :]        # [D1, N] masked
    bsel_all = head_b[first_target]

    in_maps = []
    for c in range(NCORES):
        sl = slice(c * TOK, (c + 1) * TOK)
        im = dict(shared)
        im["wiT"] = _tile_k(wiT[:, sl])
        im["selH"] = _tile_k(selH_all[:, sl])
        im["sel0"] = _tile_k(sel0_all[:, sl])
        im["sel1"] = _tile_k(sel1_all[:, sl])
        im["bsel"] = bsel_all[sl][None, :].astype(BF16)
        im["m0"] = np.ascontiguousarray(
            m0[sl].astype(np.float32).reshape(NT, 128).T
        )
        im["m1"] = np.ascontiguousarray(
            m1[sl].astype(np.float32).reshape(NT, 128).T
        )
        in_maps.append(im)

    if "nc" not in _CACHE:
        _CACHE["nc"] = _build()
    nc = _CACHE["nc"]

    trace = bool(os.environ.get("BASS_TRACE"))
    res = bass_utils.run_bass_kernel_spmd(
        nc, in_maps, core_ids=list(range(NCORES)), trace=trace
    )
    LAST_EXEC_NS = res.exec_time_ns
    total = sum(float(res.results[c]["out"][0, 0]) for c in range(NCORES))
    return np.float32(total / N)


# revision 7
# speedup vs baseline: 1.0103x; 1.0103x over previous
"""Adaptive-softmax NLL on 8 TRN2 NeuronCores (Bass/Tile, SPMD data-parallel).

Strategy: shard the 4096 tokens across 8 cores (512 each). Each core computes
its tokens' full NLL (head + both tails) in bf16 on TensorE, with vocab on the
free dim and tokens on PSUM partitions; ScalarE does exp with fused free-dim
accumulation (accum_out) so the softmax denominators come out of the same pass.
Target logits are computed from host-gathered weight columns (MoE-style
dispatch done at input-prep time) as elementwise-mul + ones-matvec partition
reductions. Each core emits one partial-loss scalar; the host sums 8 scalars
and divides by N.
"""

import os
import sys
import types

import numpy as np
import ml_dtypes

BF16 = ml_dtypes.bfloat16

# ---- problem constants (hardcoded; kernel.py must be self-contained) ----
CUTOFF = [4000, 20000, 50000]
D = 1024
N = 4096
NCORES = 8
TOK = N // NCORES          # 512 tokens per core
NT = TOK // 128            # 4 token tiles of 128
HEAD_V = CUTOFF[0] + 2     # 4002
T0_V = CUTOFF[1] - CUTOFF[0]   # 16000
T1_V = CUTOFF[2] - CUTOFF[1]   # 30000
D1 = D // 4                # 256 tail1 bottleneck


def _chunks(v):
    out = []
    while v > 0:
        out.append(min(512, v))
        v -= out[-1]
    return out


H_CH = _chunks(HEAD_V)     # [512]*7 + [418]
T0_CH = _chunks(T0_V)      # [512]*31 + [128]
T1_CH = _chunks(T1_V)      # [512]*58 + [296]

LAST_EXEC_NS = None
_CACHE = {}


def _install_axon_profile_shim():
    """The image's antenv lacks axon_hooks; register the NTFF hook + disable
    the FishPath artifact upload so BASS_TRACE=1 profiling works locally."""
    if "antenv.axon_hooks" not in sys.modules:
        try:
            import antenv  # noqa
            mod = types.ModuleType("antenv.axon_hooks")
            _hook = [None]
            mod.set_axon_ntff_profile_hook = lambda h: _hook.__setitem__(0, h)
            mod.get_axon_ntff_profile_hook = lambda: _hook[0]
            sys.modules["antenv.axon_hooks"] = mod
            antenv.axon_hooks = mod
            from trn_agent_boot.trn_boot import _ntff_profile_via_ctypes
            mod.set_axon_ntff_profile_hook(
                _ntff_profile_via_ctypes("/opt/axon/libaxon_pjrt.so")
            )
        except Exception:
            pass
    try:
        from concourse import bass_utils
        bass_utils.upload_artifacts = lambda tmpdir: f"local:{tmpdir}"
    except Exception:
        pass


# ---------------- host-side layout helpers ----------------

def _tile_k(w):
    """[K, M] f32 -> [128, K//128, M] bf16 (partition, k-tile, free)."""
    K, M = w.shape
    kd = K // 128
    return np.ascontiguousarray(
        w.reshape(kd, 128, M).transpose(1, 0, 2)
    ).astype(BF16)


def _chunk_weights(w, chunk_sizes):
    """[K, V] f32 -> [nchunk, 128, K//128, 512] bf16, zero-padded ragged."""
    K, V = w.shape
    kd = K // 128
    out = np.zeros((len(chunk_sizes), 128, kd, 512), dtype=BF16)
    c0 = 0
    for i, ncs in enumerate(chunk_sizes):
        blk = w[:, c0:c0 + ncs].reshape(kd, 128, ncs).transpose(1, 0, 2)
        out[i, :, :, :ncs] = blk.astype(BF16)
        c0 += ncs
    return out


# ---------------- device kernel builder ----------------

def _build():
    from concourse import bass, bacc, tile

    mybir = bass.mybir
    dt = mybir.dt
    bf = dt.bfloat16
    f32 = dt.float32
    AF = mybir.ActivationFunctionType
    ALU = mybir.AluOpType
    AX = mybir.AxisListType

    nc = bacc.Bacc(
        "TRN2",
        target_bir_lowering=False,
        debug=False,
        enable_asserts=False,
        num_devices=NCORES,
    )

    def din(name, shape, dtype=bf):
        return nc.dram_tensor(name, list(shape), dtype, kind="ExternalInput")

    wiT_h = din("wiT", (128, 8, TOK))
    selH_h = din("selH", (128, 8, TOK))
    sel0_h = din("sel0", (128, 8, TOK))
    sel1_h = din("sel1", (128, 2, TOK))
    bsel_h = din("bsel", (1, TOK))
    m0_h = din("m0", (128, NT), f32)
    m1_h = din("m1", (128, NT), f32)
    bext_h = din("bext", (1, HEAD_V))
    hw_h = din("hw", (len(H_CH), 128, 8, 512))
    w20_h = din("w20", (len(T0_CH), 128, 8, 512))
    w21_h = din("w21", (len(T1_CH), 128, 2, 512))
    w10_h = din("w10", (128, 8, D))
    w11_h = din("w11", (128, 8, D1))
    out_h = nc.dram_tensor("out", [1, 1], f32, kind="ExternalOutput")

    with tile.TileContext(nc) as tc:
        with (
            tc.tile_pool(name="const", bufs=1) as cpool,
            tc.tile_pool(name="wstream", bufs=4) as wpool,
            tc.tile_pool(name="scratch", bufs=3) as spool,
            tc.tile_pool(name="pmm", bufs=2, space=bass.MemorySpace.PSUM) as pmm,
        ):
            GW = 2048  # psum macro-tile width: 4 chunks x 512 = 4 banks

            def groups(chunk_sizes):
                """[(g, [(c, ncs, col_off)...], group_width)]"""
                out = []
                for g0 in range(0, len(chunk_sizes), 4):
                    cs = chunk_sizes[g0:g0 + 4]
                    items = []
                    off = 0
                    for i, ncs in enumerate(cs):
                        items.append((g0 + i, ncs, off))
                        off += ncs
                    out.append((g0 // 4, items, off))
                return out

            wiT = cpool.tile([128, 8, TOK], bf)
            w10 = cpool.tile([128, 8, D], bf)
            w11 = cpool.tile([128, 8, D1], bf)
            selH = cpool.tile([128, 8, TOK], bf)
            sel0 = cpool.tile([128, 8, TOK], bf)
            sel1 = cpool.tile([128, 2, TOK], bf)
            bsel = cpool.tile([1, TOK], bf)
            m0sb = cpool.tile([128, NT], f32)
            m1sb = cpool.tile([128, NT], f32)
            bext = cpool.tile([1, HEAD_V], bf)
            h0T = cpool.tile([128, 8, TOK], bf)
            h1T = cpool.tile([128, 2, TOK], bf)
            nGH = (len(H_CH) + 3) // 4
            nG0 = (len(T0_CH) + 3) // 4
            nG1 = (len(T1_CH) + 3) // 4
            seH = cpool.tile([128, NT, nGH], f32)
            se0 = cpool.tile([128, NT, nG0], f32)
            se1 = cpool.tile([128, NT, nG1], f32)
            ones_col = cpool.tile([128, 1], bf)
            ones_row = cpool.tile([1, 128], bf)
            onesf_col = cpool.tile([128, 1], f32)

            # resident loads (weights first; sel* only needed at the end)
            nc.sync.dma_start(out=wiT[:], in_=wiT_h[:])
            nc.sync.dma_start(out=w10[:], in_=w10_h[:])
            nc.sync.dma_start(out=w11[:], in_=w11_h[:])
            nc.sync.dma_start(out=bext[:], in_=bext_h[:])
            nc.sync.dma_start(out=m0sb[:], in_=m0_h[:])
            nc.sync.dma_start(out=m1sb[:], in_=m1_h[:])
            nc.sync.dma_start(out=bsel[:], in_=bsel_h[:])
            nc.vector.memset(ones_col[:], 1.0)
            nc.vector.memset(ones_row[:], 1.0)
            nc.vector.memset(onesf_col[:], 1.0)

            def softmax_block(wh, nk, lhsT, chunk_sizes, se, bias=None, tag="w8"):
                base = [0]
                for ncs in chunk_sizes:
                    base.append(base[-1] + ncs)
                for g, items, gw in groups(chunk_sizes):
                    wts = []
                    for c, ncs, off in items:
                        wt = wpool.tile([128, nk, 512], bf, tag=tag)
                        nc.sync.dma_start(out=wt[:], in_=wh.ap()[c])
                        wts.append(wt)
                    for jt in range(NT):
                        ps = pmm.tile([128, GW], f32, tag="mm")
                        for (c, ncs, off), wt in zip(items, wts):
                            for k in range(nk):
                                nc.tensor.matmul(
                                    ps[:, off:off + ncs],
                                    lhsT[:, k, jt * 128:(jt + 1) * 128],
                                    wt[:, k, :ncs],
                                    start=(k == 0),
                                    stop=(k == nk - 1 and bias is None),
                                )
                            if bias is not None:
                                nc.tensor.matmul(
                                    ps[:, off:off + ncs],
                                    ones_row[:, :],
                                    bias[:, base[c]:base[c] + ncs],
                                    start=False,
                                    stop=True,
                                )
                        nc.scalar.activation(
                            ps[:, :gw],
                            ps[:, :gw],
                            AF.Exp,
                            accum_out=se[:, jt, g:g + 1],
                        )

            # head CE over remapped targets (uses w_in directly)
            softmax_block(hw_h, 8, wiT, H_CH, seH, bias=bext)

            # h0T = (w_in @ tail0_w1)^T  [1024, 512] bf16
            for m in range(8):
                ps = pmm.tile([128, GW], f32, tag="mm")
                for k in range(8):
                    nc.tensor.matmul(
                        ps[:, :TOK],
                        w10[:, k, m * 128:(m + 1) * 128],
                        wiT[:, k, :],
                        start=(k == 0),
                        stop=(k == 7),
                    )
                nc.vector.tensor_copy(h0T[:, m, :], ps[:, :TOK])

            softmax_block(w20_h, 8, h0T, T0_CH, se0)

            # h1T = (w_in @ tail1_w1)^T  [256, 512] bf16
            for m in range(2):
                ps = pmm.tile([128, GW], f32, tag="mm")
                for k in range(8):
                    nc.tensor.matmul(
                        ps[:, :TOK],
                        w11[:, k, m * 128:(m + 1) * 128],
                        wiT[:, k, :],
                        start=(k == 0),
                        stop=(k == 7),
                    )
                nc.vector.tensor_copy(h1T[:, m, :], ps[:, :TOK])

            softmax_block(w21_h, 2, h1T, T1_CH, se1, tag="w2")

            # late resident loads for the target-logit dot products
            nc.sync.dma_start(out=selH[:], in_=selH_h[:])
            nc.sync.dma_start(out=sel0[:], in_=sel0_h[:])
            nc.sync.dma_start(out=sel1[:], in_=sel1_h[:])

            # target logits: sum_d a[d,t]*sel[d,t] via DVE mul + ones matvec,
            # all accumulated into one PSUM row [1, TOK]
            rowp = pmm.tile([128, GW], f32, tag="mm")
            pieces = [(wiT, selH, 8), (h0T, sel0, 8), (h1T, sel1, 2)]
            total = sum(nk for _, _, nk in pieces)
            i = 0
            for a, b, nk in pieces:
                for k in range(nk):
                    mt = spool.tile([128, TOK], bf, tag="mul")
                    nc.vector.tensor_mul(mt[:], a[:, k, :], b[:, k, :])
                    nc.tensor.matmul(
                        rowp[0:1, :TOK], ones_col[:], mt[:], start=(i == 0), stop=False
                    )
                    i += 1
            nc.tensor.matmul(
                rowp[0:1, :TOK], ones_row[:, 0:1], bsel[:], start=False, stop=True
            )

            # per-token log-sum-exp -> masked sum
            seH_r = cpool.tile([128, NT], f32)
            se0_r = cpool.tile([128, NT], f32)
            se1_r = cpool.tile([128, NT], f32)
            nc.vector.tensor_reduce(seH_r[:], seH[:], AX.X, ALU.add)
            nc.vector.tensor_reduce(se0_r[:], se0[:], AX.X, ALU.add)
            nc.vector.tensor_reduce(se1_r[:], se1[:], AX.X, ALU.add)
            logH = cpool.tile([128, NT], f32)
            log0 = cpool.tile([128, NT], f32)
            log1 = cpool.tile([128, NT], f32)
            nc.scalar.activation(logH[:], seH_r[:], AF.Ln)
            nc.scalar.activation(log0[:], se0_r[:], AF.Ln)
            nc.scalar.activation(log1[:], se1_r[:], AF.Ln)
            log0m = cpool.tile([128, NT], f32)
            log1m = cpool.tile([128, NT], f32)
            nc.vector.tensor_mul(log0m[:], log0[:], m0sb[:])
            nc.vector.tensor_mul(log1m[:], log1[:], m1sb[:])
            acc = cpool.tile([128, NT], f32)
            nc.vector.tensor_add(acc[:], logH[:], log0m[:])
            nc.vector.tensor_add(acc[:], acc[:], log1m[:])

            pf = pmm.tile([128, GW], f32, tag="mm")
            nc.tensor.matmul(pf[0:1, :NT], onesf_col[:], acc[:], start=True, stop=True)

            logsum = cpool.tile([1, 1], f32)
            tgts = cpool.tile([1, 1], f32)
            res = cpool.tile([1, 1], f32)
            nc.vector.tensor_reduce(logsum[:], pf[0:1, :NT], AX.X, ALU.add)
            nc.vector.tensor_reduce(tgts[:], rowp[0:1, :TOK], AX.X, ALU.add)
            nc.vector.tensor_sub(res[:], logsum[:], tgts[:])
            nc.sync.dma_start(out=out_h[:], in_=res[:])

    nc.compile()
    return nc


# ---------------- entry point ----------------

def kernel(**inputs):
    global LAST_EXEC_NS
    _install_axon_profile_shim()
    from concourse import bass_utils

    w_in = np.asarray(inputs["w_in"], dtype=np.float32)
    target = np.asarray(inputs["target"], dtype=np.int64)
    head_w = np.asarray(inputs["head_w"], dtype=np.float32)
    head_b = np.asarray(inputs["head_b"], dtype=np.float32)
    t0w1 = np.asarray(inputs["tail0_w1"], dtype=np.float32)
    t0w2 = np.asarray(inputs["tail0_w2"], dtype=np.float32)
    t1w1 = np.asarray(inputs["tail1_w1"], dtype=np.float32)
    t1w2 = np.asarray(inputs["tail1_w2"], dtype=np.float32)

    # target-derived bookkeeping (pure indexing, part of input sharding)
    m0 = (target >= CUTOFF[0]) & (target < CUTOFF[1])
    m1 = (target >= CUTOFF[1]) & (target < CUTOFF[2])
    first_target = np.where(m0, CUTOFF[0], np.where(m1, CUTOFF[0] + 1, target))
    idx0 = np.clip(target - CUTOFF[0], 0, T0_V - 1)
    idx1 = np.clip(target - CUTOFF[1], 0, T1_V - 1)

    # shared (replicated) weight payloads, laid out as their SBUF images
    shared = {
        "bext": head_b[None, :].astype(BF16),
        "hw": _chunk_weights(head_w, H_CH),
        "w20": _chunk_weights(t0w2, T0_CH),
        "w21": _chunk_weights(t1w2, T1_CH),
        "w10": _tile_k(t0w1),
        "w11": _tile_k(t1w1),
    }

    wiT = w_in.T  # [D, N]
    selH_all = head_w[:, first_target]            # [D, N]
    sel0_all = t0w2[:, idx0] * m0[None, :]        # [D, N] masked
    sel1_all = t1w2[:, idx1] * m1[None, :]        # [D1, N] masked
    bsel_all = head_b[first_target]

    in_maps = []
    for c in range(NCORES):
        sl = slice(c * TOK, (c + 1) * TOK)
        im = dict(shared)
        im["wiT"] = _tile_k(wiT[:, sl])
        im["selH"] = _tile_k(selH_all[:, sl])
        im["sel0"] = _tile_k(sel0_all[:, sl])
        im["sel1"] = _tile_k(sel1_all[:, sl])
        im["bsel"] = bsel_all[sl][None, :].astype(BF16)
        im["m0"] = np.ascontiguousarray(
            m0[sl].astype(np.float32).reshape(NT, 128).T
        )
        im["m1"] = np.ascontiguousarray(
            m1[sl].astype(np.float32).reshape(NT, 128).T
        )
        in_maps.append(im)

    if "nc" not in _CACHE:
        _CACHE["nc"] = _build()
    nc = _CACHE["nc"]

    trace = bool(os.environ.get("BASS_TRACE"))
    res = bass_utils.run_bass_kernel_spmd(
        nc, in_maps, core_ids=list(range(NCORES)), trace=trace
    )
    LAST_EXEC_NS = res.exec_time_ns
    total = sum(float(res.results[c]["out"][0, 0]) for c in range(NCORES))
    return np.float32(total / N)


# revision 8
# speedup vs baseline: 1.6142x; 1.5977x over previous
"""Adaptive-softmax NLL on 8 TRN2 NeuronCores (Bass/Tile, SPMD data-parallel).

Strategy: shard the 4096 tokens across 8 cores (512 each). Each core computes
its tokens' full NLL (head + both tails) in bf16 on TensorE, with vocab on the
free dim and tokens on PSUM partitions; ScalarE does exp with fused free-dim
accumulation (accum_out) so the softmax denominators come out of the same pass.
Target logits are computed from host-gathered weight columns (MoE-style
dispatch done at input-prep time) as elementwise-mul + ones-matvec partition
reductions. Each core emits one partial-loss scalar; the host sums 8 scalars
and divides by N.
"""

import os
import sys
import types

import numpy as np
import ml_dtypes

BF16 = ml_dtypes.bfloat16
FP8 = ml_dtypes.float8_e4m3
W8_SCALE = 256.0

# ---- problem constants (hardcoded; kernel.py must be self-contained) ----
CUTOFF = [4000, 20000, 50000]
D = 1024
N = 4096
NCORES = 8
TOK = N // NCORES          # 512 tokens per core
NT = TOK // 128            # 4 token tiles of 128
HEAD_V = CUTOFF[0] + 2     # 4002
T0_V = CUTOFF[1] - CUTOFF[0]   # 16000
T1_V = CUTOFF[2] - CUTOFF[1]   # 30000
D1 = D // 4                # 256 tail1 bottleneck


def _chunks(v):
    out = []
    while v > 0:
        out.append(min(512, v))
        v -= out[-1]
    return out


H_CH = _chunks(HEAD_V)     # [512]*7 + [418]
T0_CH = _chunks(T0_V)      # [512]*31 + [128]
T1_CH = _chunks(T1_V)      # [512]*58 + [296]

LAST_EXEC_NS = None
_CACHE = {}


def _install_axon_profile_shim():
    """The image's antenv lacks axon_hooks; register the NTFF hook + disable
    the FishPath artifact upload so BASS_TRACE=1 profiling works locally."""
    if "antenv.axon_hooks" not in sys.modules:
        try:
            import antenv  # noqa
            mod = types.ModuleType("antenv.axon_hooks")
            _hook = [None]
            mod.set_axon_ntff_profile_hook = lambda h: _hook.__setitem__(0, h)
            mod.get_axon_ntff_profile_hook = lambda: _hook[0]
            sys.modules["antenv.axon_hooks"] = mod
            antenv.axon_hooks = mod
            from trn_agent_boot.trn_boot import _ntff_profile_via_ctypes
            mod.set_axon_ntff_profile_hook(
                _ntff_profile_via_ctypes("/opt/axon/libaxon_pjrt.so")
            )
        except Exception:
            pass
    try:
        from concourse import bass_utils
        bass_utils.upload_artifacts = lambda tmpdir: f"local:{tmpdir}"
    except Exception:
        pass


# ---------------- host-side layout helpers ----------------

def _tile_k(w):
    """[K, M] f32 -> [128, K//128, M] bf16 (partition, k-tile, free)."""
    K, M = w.shape
    kd = K // 128
    return np.ascontiguousarray(
        w.reshape(kd, 128, M).transpose(1, 0, 2)
    ).astype(BF16)


def _chunk_weights(w, chunk_sizes, dtype=BF16, scale=1.0):
    """[K, V] f32 -> [nchunk, 128, K//128, 512], zero-padded ragged."""
    K, V = w.shape
    kd = K // 128
    out = np.zeros((len(chunk_sizes), 128, kd, 512), dtype=dtype)
    c0 = 0
    for i, ncs in enumerate(chunk_sizes):
        blk = (w[:, c0:c0 + ncs] * scale).reshape(kd, 128, ncs).transpose(1, 0, 2)
        out[i, :, :, :ncs] = blk.astype(dtype)
        c0 += ncs
    return out


# ---------------- device kernel builder ----------------

def _build():
    from concourse import bass, bacc, tile  # noqa: W8_SCALE from module scope

    mybir = bass.mybir
    dt = mybir.dt
    bf = dt.bfloat16
    f32 = dt.float32
    AF = mybir.ActivationFunctionType
    ALU = mybir.AluOpType
    AX = mybir.AxisListType

    nc = bacc.Bacc(
        "TRN2",
        target_bir_lowering=False,
        debug=False,
        enable_asserts=False,
        num_devices=NCORES,
    )

    def din(name, shape, dtype=bf):
        return nc.dram_tensor(name, list(shape), dtype, kind="ExternalInput")

    f8 = dt.float8e4
    wiT_h = din("wiT", (128, 8, TOK))
    wiT8_h = din("wiT8", (128, 8, TOK), f8)
    selH_h = din("selH", (128, 8, TOK))
    sel0_h = din("sel0", (128, 8, TOK))
    sel1_h = din("sel1", (128, 2, TOK))
    bsel_h = din("bsel", (1, TOK))
    m0_h = din("m0", (128, NT), f32)
    m1_h = din("m1", (128, NT), f32)
    bext_h = din("bext", (1, HEAD_V))
    hw_h = din("hw", (len(H_CH), 128, 8, 512), f8)
    w20_h = din("w20", (len(T0_CH), 128, 8, 512), f8)
    w21_h = din("w21", (len(T1_CH), 128, 2, 512), f8)
    w10_h = din("w10", (128, 8, D))
    w11_h = din("w11", (128, 8, D1))
    out_h = nc.dram_tensor("out", [1, 1], f32, kind="ExternalOutput")

    with tile.TileContext(nc) as tc:
        with (
            tc.tile_pool(name="const", bufs=1) as cpool,
            tc.tile_pool(name="wstream", bufs=4) as wpool,
            tc.tile_pool(name="scratch", bufs=3) as spool,
            tc.tile_pool(name="pmm", bufs=2, space=bass.MemorySpace.PSUM) as pmm,
        ):
            GW = 2048  # psum macro-tile width: 4 chunks x 512 = 4 banks

            def groups(chunk_sizes):
                """[(g, [(c, ncs, col_off)...], group_width)]"""
                out = []
                for g0 in range(0, len(chunk_sizes), 4):
                    cs = chunk_sizes[g0:g0 + 4]
                    items = []
                    off = 0
                    for i, ncs in enumerate(cs):
                        items.append((g0 + i, ncs, off))
                        off += ncs
                    out.append((g0 // 4, items, off))
                return out

            wiT = cpool.tile([128, 8, TOK], bf)
            w10 = cpool.tile([128, 8, D], bf)
            w11 = cpool.tile([128, 8, D1], bf)
            selH = cpool.tile([128, 8, TOK], bf)
            sel0 = cpool.tile([128, 8, TOK], bf)
            sel1 = cpool.tile([128, 2, TOK], bf)
            bsel = cpool.tile([1, TOK], bf)
            m0sb = cpool.tile([128, NT], f32)
            m1sb = cpool.tile([128, NT], f32)
            bext = cpool.tile([1, HEAD_V], bf)
            h0T = cpool.tile([128, 8, TOK], bf)
            h1T = cpool.tile([128, 2, TOK], bf)
            wiT8 = cpool.tile([128, 8, TOK], f8)
            h0T8 = cpool.tile([128, 8, TOK], f8)
            h1T8 = cpool.tile([128, 2, TOK], f8)
            nGH = (len(H_CH) + 3) // 4
            nG0 = (len(T0_CH) + 3) // 4
            nG1 = (len(T1_CH) + 3) // 4
            seH = cpool.tile([128, NT, nGH], f32)
            se0 = cpool.tile([128, NT, nG0], f32)
            se1 = cpool.tile([128, NT, nG1], f32)
            ones_col = cpool.tile([128, 1], bf)
            ones_row = cpool.tile([1, 128], bf)
            onesf_col = cpool.tile([128, 1], f32)

            # resident loads (weights first; sel* only needed at the end)
            nc.sync.dma_start(out=wiT[:], in_=wiT_h[:])
            nc.sync.dma_start(out=wiT8[:], in_=wiT8_h[:])
            nc.sync.dma_start(out=w10[:], in_=w10_h[:])
            nc.sync.dma_start(out=w11[:], in_=w11_h[:])
            nc.sync.dma_start(out=bext[:], in_=bext_h[:])
            nc.sync.dma_start(out=m0sb[:], in_=m0_h[:])
            nc.sync.dma_start(out=m1sb[:], in_=m1_h[:])
            nc.sync.dma_start(out=bsel[:], in_=bsel_h[:])
            nc.vector.memset(ones_col[:], 1.0)
            nc.vector.memset(ones_row[:], 1.0)
            nc.vector.memset(onesf_col[:], 1.0)

            DR = mybir.MatmulPerfMode.DoubleRow

            def softmax_block(wh, nk, lhsT8, chunk_sizes, se, bias=None, tag="w8"):
                base = [0]
                for ncs in chunk_sizes:
                    base.append(base[-1] + ncs)
                nk2 = nk // 2
                for g, items, gw in groups(chunk_sizes):
                    wts = []
                    for c, ncs, off in items:
                        wt = wpool.tile([128, nk, 512], f8, tag=tag)
                        nc.sync.dma_start(out=wt[:], in_=wh.ap()[c])
                        wts.append(wt)
                    for jt in range(NT):
                        ps = pmm.tile([128, GW], f32, tag="mm")
                        for (c, ncs, off), wt in zip(items, wts):
                            for k2 in range(nk2):
                                nc.tensor.matmul(
                                    ps[:, off:off + ncs],
                                    lhsT8[:, 2 * k2:2 * k2 + 2,
                                          jt * 128:(jt + 1) * 128],
                                    wt[:, 2 * k2:2 * k2 + 2, :ncs],
                                    start=(k2 == 0),
                                    stop=(k2 == nk2 - 1 and bias is None),
                                    perf_mode=DR,
                                )
                            if bias is not None:
                                nc.tensor.matmul(
                                    ps[:, off:off + ncs],
                                    ones_row[:, :],
                                    bias[:, base[c]:base[c] + ncs],
                                    start=False,
                                    stop=True,
                                )
                        nc.scalar.activation(
                            ps[:, :gw],
                            ps[:, :gw],
                            AF.Exp,
                            scale=1.0 / W8_SCALE,
                            accum_out=se[:, jt, g:g + 1],
                        )

            # head CE over remapped targets (uses w_in directly)
            softmax_block(hw_h, 8, wiT8, H_CH, seH, bias=bext)

            # h0T = (w_in @ tail0_w1)^T  [1024, 512] bf16
            for m in range(8):
                ps = pmm.tile([128, GW], f32, tag="mm")
                for k in range(8):
                    nc.tensor.matmul(
                        ps[:, :TOK],
                        w10[:, k, m * 128:(m + 1) * 128],
                        wiT[:, k, :],
                        start=(k == 0),
                        stop=(k == 7),
                    )
                nc.vector.tensor_copy(h0T[:, m, :], ps[:, :TOK])
                nc.vector.tensor_copy(h0T8[:, m, :], ps[:, :TOK])

            softmax_block(w20_h, 8, h0T8, T0_CH, se0)

            # h1T = (w_in @ tail1_w1)^T  [256, 512] bf16
            for m in range(2):
                ps = pmm.tile([128, GW], f32, tag="mm")
                for k in range(8):
                    nc.tensor.matmul(
                        ps[:, :TOK],
                        w11[:, k, m * 128:(m + 1) * 128],
                        wiT[:, k, :],
                        start=(k == 0),
                        stop=(k == 7),
                    )
                nc.vector.tensor_copy(h1T[:, m, :], ps[:, :TOK])
                nc.vector.tensor_copy(h1T8[:, m, :], ps[:, :TOK])

            softmax_block(w21_h, 2, h1T8, T1_CH, se1, tag="w2")

            # late resident loads for the target-logit dot products
            nc.sync.dma_start(out=selH[:], in_=selH_h[:])
            nc.sync.dma_start(out=sel0[:], in_=sel0_h[:])
            nc.sync.dma_start(out=sel1[:], in_=sel1_h[:])

            # target logits: sum_d a[d,t]*sel[d,t] via DVE mul + ones matvec,
            # all accumulated into one PSUM row [1, TOK]
            rowp = pmm.tile([128, GW], f32, tag="mm")
            pieces = [(wiT, selH, 8), (h0T, sel0, 8), (h1T, sel1, 2)]
            total = sum(nk for _, _, nk in pieces)
            i = 0
            for a, b, nk in pieces:
                for k in range(nk):
                    mt = spool.tile([128, TOK], bf, tag="mul")
                    nc.vector.tensor_mul(mt[:], a[:, k, :], b[:, k, :])
                    nc.tensor.matmul(
                        rowp[0:1, :TOK], ones_col[:], mt[:], start=(i == 0), stop=False
                    )
                    i += 1
            nc.tensor.matmul(
                rowp[0:1, :TOK], ones_row[:, 0:1], bsel[:], start=False, stop=True
            )

            # per-token log-sum-exp -> masked sum
            seH_r = cpool.tile([128, NT], f32)
            se0_r = cpool.tile([128, NT], f32)
            se1_r = cpool.tile([128, NT], f32)
            nc.vector.tensor_reduce(seH_r[:], seH[:], AX.X, ALU.add)
            nc.vector.tensor_reduce(se0_r[:], se0[:], AX.X, ALU.add)
            nc.vector.tensor_reduce(se1_r[:], se1[:], AX.X, ALU.add)
            logH = cpool.tile([128, NT], f32)
            log0 = cpool.tile([128, NT], f32)
            log1 = cpool.tile([128, NT], f32)
            nc.scalar.activation(logH[:], seH_r[:], AF.Ln)
            nc.scalar.activation(log0[:], se0_r[:], AF.Ln)
            nc.scalar.activation(log1[:], se1_r[:], AF.Ln)
            log0m = cpool.tile([128, NT], f32)
            log1m = cpool.tile([128, NT], f32)
            nc.vector.tensor_mul(log0m[:], log0[:], m0sb[:])
            nc.vector.tensor_mul(log1m[:], log1[:], m1sb[:])
            acc = cpool.tile([128, NT], f32)
            nc.vector.tensor_add(acc[:], logH[:], log0m[:])
            nc.vector.tensor_add(acc[:], acc[:], log1m[:])

            pf = pmm.tile([128, GW], f32, tag="mm")
            nc.tensor.matmul(pf[0:1, :NT], onesf_col[:], acc[:], start=True, stop=True)

            logsum = cpool.tile([1, 1], f32)
            tgts = cpool.tile([1, 1], f32)
            res = cpool.tile([1, 1], f32)
            nc.vector.tensor_reduce(logsum[:], pf[0:1, :NT], AX.X, ALU.add)
            nc.vector.tensor_reduce(tgts[:], rowp[0:1, :TOK], AX.X, ALU.add)
            nc.vector.tensor_sub(res[:], logsum[:], tgts[:])
            nc.sync.dma_start(out=out_h[:], in_=res[:])

    nc.compile()
    return nc


# ---------------- entry point ----------------

def kernel(**inputs):
    global LAST_EXEC_NS
    _install_axon_profile_shim()
    from concourse import bass_utils

    w_in = np.asarray(inputs["w_in"], dtype=np.float32)
    target = np.asarray(inputs["target"], dtype=np.int64)
    head_w = np.asarray(inputs["head_w"], dtype=np.float32)
    head_b = np.asarray(inputs["head_b"], dtype=np.float32)
    t0w1 = np.asarray(inputs["tail0_w1"], dtype=np.float32)
    t0w2 = np.asarray(inputs["tail0_w2"], dtype=np.float32)
    t1w1 = np.asarray(inputs["tail1_w1"], dtype=np.float32)
    t1w2 = np.asarray(inputs["tail1_w2"], dtype=np.float32)

    # target-derived bookkeeping (pure indexing, part of input sharding)
    m0 = (target >= CUTOFF[0]) & (target < CUTOFF[1])
    m1 = (target >= CUTOFF[1]) & (target < CUTOFF[2])
    first_target = np.where(m0, CUTOFF[0], np.where(m1, CUTOFF[0] + 1, target))
    idx0 = np.clip(target - CUTOFF[0], 0, T0_V - 1)
    idx1 = np.clip(target - CUTOFF[1], 0, T1_V - 1)

    # shared (replicated) weight payloads, laid out as their SBUF images
    shared = {
        "bext": (head_b[None, :] * W8_SCALE).astype(BF16),
        "hw": _chunk_weights(head_w, H_CH, FP8, W8_SCALE),
        "w20": _chunk_weights(t0w2, T0_CH, FP8, W8_SCALE),
        "w21": _chunk_weights(t1w2, T1_CH, FP8, W8_SCALE),
        "w10": _tile_k(t0w1),
        "w11": _tile_k(t1w1),
    }

    wiT = w_in.T  # [D, N]
    selH_all = head_w[:, first_target]            # [D, N]
    sel0_all = t0w2[:, idx0] * m0[None, :]        # [D, N] masked
    sel1_all = t1w2[:, idx1] * m1[None, :]        # [D1, N] masked
    bsel_all = head_b[first_target]

    in_maps = []
    for c in range(NCORES):
        sl = slice(c * TOK, (c + 1) * TOK)
        im = dict(shared)
        im["wiT"] = _tile_k(wiT[:, sl])
        im["wiT8"] = _tile_k(wiT[:, sl]).astype(FP8)
        im["selH"] = _tile_k(selH_all[:, sl])
        im["sel0"] = _tile_k(sel0_all[:, sl])
        im["sel1"] = _tile_k(sel1_all[:, sl])
        im["bsel"] = bsel_all[sl][None, :].astype(BF16)
        im["m0"] = np.ascontiguousarray(
            m0[sl].astype(np.float32).reshape(NT, 128).T
        )
        im["m1"] = np.ascontiguousarray(
            m1[sl].astype(np.float32).reshape(NT, 128).T
        )
        in_maps.append(im)

    if "nc" not in _CACHE:
        _CACHE["nc"] = _build()
    nc = _CACHE["nc"]

    trace = bool(os.environ.get("BASS_TRACE"))
    res = bass_utils.run_bass_kernel_spmd(
        nc, in_maps, core_ids=list(range(NCORES)), trace=trace
    )
    LAST_EXEC_NS = res.exec_time_ns
    total = sum(float(res.results[c]["out"][0, 0]) for c in range(NCORES))
    return np.float32(total / N)
